# revision 11
# baseline (speedup 1.0000x reference)
"""Trainium2 Bass kernel for nn_C2M_24378234372461.

Data-parallel over batch (8 samples on 8 NeuronCores). BatchNorm batch
statistics are exchanged with two small AllReduce collectives.
Self-contained: builds + compiles the Bass program on first call.
"""
import sys

for _p in ("/opt/trn_rl_repo",):
    if _p not in sys.path:
        sys.path.append(_p)

import numpy as np
import concourse.bacc as bacc
import concourse.bass as bass
import concourse.mybir as mybir
import concourse.tile as tile
import concourse.masks as masks
from concourse.bass_utils import run_bass_kernel_spmd

f32 = mybir.dt.float32
f32r = mybir.dt.float32r
bf16 = mybir.dt.bfloat16
AF = mybir.ActivationFunctionType
AX = mybir.AxisListType
ALU = mybir.AluOpType

N_CORES = 8
B = 8
C2, H2, W2 = 128, 44, 44
C3, H3, W3 = 256, 22, 22
C4, H4, W4 = 512, 11, 11
HW2 = H2 * W2            # 1936
HW3 = H3 * W3            # 484
HW4 = H4 * W4            # 121
PG2 = 46 * 46            # 2116 padded grid scale-2
PG3 = 24 * 24            # 576  padded grid scale-3
XP2_W = PG2 + 96         # flat buffer + tail for overcompute reads (2212)
XP3_W = PG3 + 50         # 626 per cin tile
EPS = 1e-5


def _view2d(ap, width):
    """[p, (rows width)] view of a flat AP."""
    return ap.rearrange("p (r w) -> p r w", w=width)


def build(shared, DEBUG=False):
    nc = bacc.Bacc("TRN2", target_bir_lowering=False, debug=False,
                   num_devices=N_CORES)

    # ---------------- DRAM I/O ----------------
    d_xp4s = nc.dram_tensor("xp4s", [4, 128, 9 * HW4], bf16, kind="ExternalInput")
    d_xp3 = nc.dram_tensor("xp3", [128, 2 * XP3_W], bf16, kind="ExternalInput")
    d_xp2 = nc.dram_tensor("xp2", [128, XP2_W], bf16, kind="ExternalInput")
    d_w4qk = nc.inline_tensor(shared["w4qk"], "w4qk")
    d_wn3 = nc.inline_tensor(shared["wn3"], "wn3")
    d_wn2 = nc.inline_tensor(shared["wn2"], "wn2")
    d_w2r = nc.inline_tensor(shared["w2r"], "w2r")
    d_w13 = nc.inline_tensor(shared["w13"], "w13")
    d_w12 = nc.inline_tensor(shared["w12"], "w12")
    d_bn4 = nc.inline_tensor(shared["bn4"], "bn4")
    d_vecs = nc.inline_tensor(shared["vecs"], "vecs")
    d_ones = nc.inline_tensor(shared["onesd"], "onesd")
    d_out = nc.dram_tensor("out", [128, HW2], f32, kind="ExternalOutput")
    if DEBUG:
        dbg_zqkT = nc.dram_tensor("dbg_zqkT", [121, 512], f32, kind="ExternalOutput")
        dbg_gstats = nc.dram_tensor("dbg_gstats", [1, 1024], f32, kind="ExternalOutput")
        dbg_r4qkT = nc.dram_tensor("dbg_r4qkT", [128, 512], f32, kind="ExternalOutput")
        dbg_r3 = nc.dram_tensor("dbg_r3", [128, 968], f32, kind="ExternalOutput")
        dbg_q3 = nc.dram_tensor("dbg_q3", [128, 484], f32, kind="ExternalOutput")
        dbg_r2 = nc.dram_tensor("dbg_r2", [128, 1936], f32, kind="ExternalOutput")
        dbg_s2 = nc.dram_tensor("dbg_s2", [128, 1938], f32, kind="ExternalOutput")
        dbg_q2 = nc.dram_tensor("dbg_q2", [128, 1934], f32, kind="ExternalOutput")
        dbg_y = nc.dram_tensor("dbg_y", [128, 1936], f32, kind="ExternalOutput")
        dbg_gs2 = nc.dram_tensor("dbg_gs2", [128, 2], f32, kind="ExternalOutput")

    # collective bounce buffers
    cc0_in = nc.dram_tensor("cc0_in", [1, 8], f32r)
    cc0_out = nc.dram_tensor("cc0_out", [1, 8], f32r, addr_space="Shared")
    cc1_in = nc.dram_tensor("cc1_in", [1, 1024], f32r)
    cc1_out = nc.dram_tensor("cc1_out", [1, 1024], f32r, addr_space="Shared")
    cc2_in = nc.dram_tensor("cc2_in", [128, 2], f32r)
    cc2_out = nc.dram_tensor("cc2_out", [128, 2], f32r, addr_space="Shared")
    RG = [list(range(N_CORES))]

    with tile.TileContext(nc) as tc:
        _build_body(nc, tc, locals())
    nc.compile()
    return nc


def _build_body(nc, tc, d):
    from contextlib import ExitStack

    ctx = ExitStack()
    with ctx:
        const = ctx.enter_context(tc.tile_pool(name="const", bufs=1))
        acts = ctx.enter_context(tc.tile_pool(name="acts", bufs=1))
        scr = ctx.enter_context(tc.tile_pool(name="scr", bufs=3))
        attp = ctx.enter_context(tc.tile_pool(name="attp", bufs=3))
        ps_tmp = ctx.enter_context(tc.tile_pool(name="ps_tmp", bufs=2, space="PSUM"))
        ps_big = ctx.enter_context(tc.tile_pool(name="ps_big", bufs=1, space="PSUM"))

        _tmp_i = [0]

        def tmp_ps(p, n):
            _tmp_i[0] += 1
            return ps_tmp.tile([p, n], f32, tag="tmp", name=f"tps{_tmp_i[0]}")

        # ------------- early PE warm-up + CC bootstrap -------------
        ident = const.tile([128, 128], f32)
        masks.make_identity(nc, ident[:])
        warm = const.tile([128, 128], bf16)
        nc.vector.memset(warm[:], 0.5)
        for wi in range(20):
            wp = tmp_ps(128, 128)
            nc.tensor.matmul(wp[:], warm[:], warm[:], start=True, stop=True)
        # dummy collective: pays the one-time CC bootstrap costs (barrier,
        # first-CC trigger delay) concurrently with input DMAs + conv4
        nc.gpsimd.collective_compute(
            "AllReduce", ALU.add, replica_groups=d["RG"],
            ins=[d["cc0_in"][:].opt()], outs=[d["cc0_out"][:].opt()])
        # preload the ln/exp activation table set (the only set we use)
        dummy_act = const.tile([1, 8], f32)
        nc.scalar.activation(dummy_act[:], ident[:1, :8], AF.Exp)
        nc.scalar.activation(dummy_act[:], ident[:1, :8], AF.Ln)

        # ------------- constants / weights (persistent) -------------
        ones_sb = const.tile([128, 128], f32r)
        nc.sync.dma_start(ones_sb[:], d["d_ones"][:].bitcast(f32r))
        vecs = const.tile([128, 10], f32)
        nc.sync.dma_start(vecs[:], d["d_vecs"][:])
        bn4gb = const.tile([1, 1024], f32)
        wn3 = const.tile([128, 4608], bf16)
        wn2 = const.tile([128, 1152], bf16)
        w2r = const.tile([128, 1152], bf16)
        w13 = const.tile([128, 1536], bf16)
        w12 = const.tile([128, 1280], bf16)

        # ------------- persistent activations -------------
        xp2 = acts.tile([128, XP2_W], bf16)
        r2 = acts.tile([128, HW2], f32r)
        q3 = acts.tile([128, HW3], f32r)
        k3T = acts.tile([128, 512], f32)
        s2pad = acts.tile([128, HW2 + 2], bf16)
        q2 = acts.tile([128, 1934], f32r)
        k2 = acts.tile([128, 1934], f32)
        k2T = acts.tile([128, 2048], f32)
        r2fpad = acts.tile([128, XP2_W], bf16)
        y_sb = acts.tile([128, HW2], f32)

        # ============ PHASE A: conv4 + stats + CC1; r3/r2 under CC1 ============
        with tc.tile_pool(name="s1", bufs=1) as s1p, \
             tc.tile_pool(name="s1s", bufs=1) as s1s:
            xp3 = s1p.tile([128, 2 * XP3_W], bf16)

            # z^T = conv(x4) for q|k stacked: [121, 512]
            zT = tmp_ps(121, 512)
            with tc.tile_pool(name="s1w", bufs=2) as s1w, \
                 tc.tile_pool(name="s1x", bufs=2) as s1x:
                for t in range(4):
                    xc = s1x.tile([128, 9 * HW4], bf16, tag="x4c",
                                  name=f"x4c{t}")
                    nc.sync.dma_start(xc[:], d["d_xp4s"][t])
                    for tp2 in range(3):
                        wc = s1w.tile([128, 1536], bf16, tag="w4c",
                                      name=f"w4c{t}_{tp2}")
                        nc.sync.dma_start(
                            wc[:],
                            d["d_w4qk"][t][:, tp2 * 1536:(tp2 + 1) * 1536])
                        for tj in range(3):
                            tap = tp2 * 3 + tj
                            nc.tensor.matmul(
                                zT[:],
                                xc[:, tap * HW4:(tap + 1) * HW4],
                                wc[:, tj * 512:(tj + 1) * 512],
                                start=(t == 0 and tap == 0),
                                stop=(t == 3 and tap == 8))
                # bulk DMAs issue from the ACT HWDGE ring, gated on the
                # last conv4 input chunk so conv4's stream gets the HBM
                # bandwidth first
                gate = const.tile([1, 8], bf16)
                nc.scalar.copy(gate[:1, :1], xc[:1, :1])
                nc.scalar.dma_start(xp3[:], d["d_xp3"][:])
                nc.scalar.dma_start(wn3[:], d["d_wn3"][:])
                nc.scalar.dma_start(xp2[:], d["d_xp2"][:])
                nc.scalar.dma_start(wn2[:], d["d_wn2"][:])
                nc.scalar.dma_start(bn4gb[:, :512], d["d_bn4"][0:1, :])
                nc.scalar.dma_start(bn4gb[:, 512:], d["d_bn4"][1:2, :])
                nc.scalar.dma_start(w13[:], d["d_w13"][:])
                nc.scalar.dma_start(w12[:], d["d_w12"][:])
                nc.scalar.dma_start(w2r[:], d["d_w2r"][:])

                # zero-fill pads on otherwise-idle engines (off critical path)
                nc.vector.memset(s2pad[:, 0:1], 0.0)
                nc.vector.memset(s2pad[:, HW2 + 1:], 0.0)
                nc.vector.memset(r2fpad[:], 0.0)

                # local BN stats for scale-4: [1, 512+512] (sum, sumsq)
                zqkT = s1p.tile([121, 512], f32r)
                nc.vector.tensor_copy(zqkT[:], zT[:])
                if d.get("DEBUG"):
                    nc.sync.dma_start(d["dbg_zqkT"][:], zqkT[:].bitcast(f32))
                zsq = s1p.tile([121, 512], f32r, tag="zt1", name="zsq")
                nc.vector.tensor_mul(zsq[:], zqkT[:], zqkT[:])
                stats_ps = tmp_ps(1, 1024)
                nc.tensor.matmul(stats_ps[:, :512], ones_sb[:121, :1], zqkT[:],
                                 start=True, stop=True)
                nc.tensor.matmul(stats_ps[:, 512:], ones_sb[:121, :1], zsq[:],
                                 start=True, stop=True)
                stats1 = s1p.tile([1, 1024], f32r)
                nc.vector.tensor_copy(stats1[:], stats_ps[:])
                nc.sync.dma_start(d["cc1_in"][:], stats1[:])
                nc.gpsimd.collective_compute(
                    "AllReduce", ALU.add, replica_groups=d["RG"],
                    ins=[d["cc1_in"][:].opt()], outs=[d["cc1_out"][:].opt()])
                gstats = s1p.tile([1, 1024], f32r)
                nc.sync.dma_start(gstats[:], d["cc1_out"][:])
                if d.get("DEBUG"):
                    nc.sync.dma_start(d["dbg_gstats"][:], gstats[:].bitcast(f32))

            # ---- r3 = conv_n3(x3) + bias (independent of CC1) ----
            r3 = s1p.tile([128, 2 * HW3], f32r)
            for ct in range(2):
                for ch in range(2):          # padded-grid chunks of 288 (12 rows)
                    pc = tmp_ps(128, 288)
                    for kt in range(2):
                        for tap in range(9):
                            dy, dx = tap // 3, tap % 3
                            off = kt * XP3_W + ch * 288 + dy * 24 + dx
                            nc.tensor.matmul(
                                pc[:], wn3[:, (kt * 18 + ct * 9 + tap) * 128:
                                           (kt * 18 + ct * 9 + tap + 1) * 128],
                                xp3[:, off: off + 288],
                                start=(kt == 0 and tap == 0),
                                stop=(kt == 1 and tap == 8))
                    # valid outputs: top-left corner rows [0,22) of grid
                    r0 = ch * 12
                    nr = min(12, 22 - r0)
                    src = _view2d(pc[:, :nr * 24], 24)[:, :, :22]
                    nc.scalar.activation(
                        r3[:, ct * HW3 + r0 * 22:
                           ct * HW3 + r0 * 22 + nr * 22],
                        src, AF.Identity, bias=vecs[:, ct:ct + 1])
            if d.get("DEBUG"):
                nc.sync.dma_start(d["dbg_r3"][:], r3[:])

            # ---- r2 = conv_n2(x2) + bias (independent of CC1) ----
            CH2 = [(0, 506), (506, 506), (1012, 506), (1518, 506)]
            for ci, (st, sz) in enumerate(CH2):
                pc = tmp_ps(128, sz)
                for tap in range(9):
                    dy, dx = tap // 3, tap % 3
                    nc.tensor.matmul(pc[:], wn2[:, tap * 128:(tap + 1) * 128],
                                     xp2[:, st + dy * 46 + dx: st + dy * 46 + dx + sz],
                                     start=(tap == 0), stop=(tap == 8))
                r0 = st // 46
                src = _view2d(pc[:, :11 * 46], 46)[:, :, :44]
                nc.scalar.activation(
                    r2[:, r0 * 44: r0 * 44 + 11 * 44], src,
                    AF.Identity, bias=vecs[:, 2:3])
            if d.get("DEBUG"):
                nc.sync.dma_start(d["dbg_r2"][:], r2[:].bitcast(f32))

            # ---- BN4 affine (needs CC1), broadcast to 121 partitions first
            # so every DVE/ACT op runs 121 lanes wide
            c1 = 1.0 / (B * HW4)
            gb = ps_big.tile([121, 2048], f32, tag="big", name="gbcast")
            for j in range(2):
                nc.tensor.matmul(gb[:, j * 512:(j + 1) * 512], ones_sb[:1, :121],
                                 gstats[:, j * 512:(j + 1) * 512],
                                 start=True, stop=True)
                nc.tensor.matmul(gb[:, 1024 + j * 512: 1536 + j * 512],
                                 ones_sb[:1, :121],
                                 bn4gb[:, j * 512:(j + 1) * 512].bitcast(f32r),
                                 start=True, stop=True)
            meanb = s1p.tile([121, 512], f32, tag="af1")
            nc.vector.tensor_scalar_mul(meanb[:], gb[:, :512], c1)
            ex2b = s1p.tile([121, 512], f32, tag="af2")
            nc.vector.tensor_scalar_mul(ex2b[:], gb[:, 512:1024], c1)
            varb = s1p.tile([121, 512], f32, tag="af3")
            nc.vector.tensor_mul(varb[:], meanb[:], meanb[:])
            nc.vector.tensor_sub(varb[:], ex2b[:], varb[:])
            epsb = s1s.tile([121, 1], f32, tag="eps1")
            nc.vector.memset(epsb[:], EPS)
            lnb = s1s.tile([121, 512], f32, tag="v4")
            nc.scalar.activation(lnb[:], varb[:], AF.Ln, bias=epsb[:])
            rstdb = s1s.tile([121, 512], f32, tag="v5")
            nc.scalar.activation(rstdb[:], lnb[:], AF.Exp, scale=-0.5)
            Ab = s1s.tile([121, 512], f32, tag="v6")
            nc.vector.tensor_mul(Ab[:], gb[:, 1024:1536], rstdb[:])
            Bb = s1s.tile([121, 512], f32, tag="v7")
            nc.vector.tensor_mul(Bb[:], meanb[:], Ab[:])
            nc.vector.tensor_sub(Bb[:], gb[:, 1536:2048], Bb[:])
            t1 = s1p.tile([121, 512], f32, tag="zt1", name="t1")
            nc.vector.tensor_mul(t1[:], zqkT[:], Ab[:])
            nc.vector.tensor_add(t1[:], t1[:], Bb[:])
            # full 128 partitions (zero rows 121..127) so PE transposes have
            # even output free size
            r4qkT = s1p.tile([128, 512], f32)
            nc.vector.memset(r4qkT[:], 0.0)
            nc.scalar.activation(r4qkT[:121, :], t1[:], AF.Relu)
            if d.get("DEBUG"):
                nc.sync.dma_start(d["dbg_r4qkT"][:], r4qkT[:])

            # att43 logits (f32): [121, 484]
            r4q = s1p.tile([128, 2 * HW4], f32r)
            for ct in range(2):
                trp = tmp_ps(128, 128)
                nc.tensor.transpose(
                    trp[:], r4qkT[:, ct * 128:(ct + 1) * 128], ident[:])
                nc.vector.tensor_copy(r4q[:, ct * HW4:(ct + 1) * HW4],
                                      trp[:, :HW4])
            l43 = tmp_ps(121, HW3)
            for ct in range(2):
                nc.tensor.matmul(l43[:], r4q[:, ct * HW4:(ct + 1) * HW4],
                                 r3[:, ct * HW3:(ct + 1) * HW3],
                                 start=(ct == 0), stop=(ct == 1))
            att43 = s1p.tile([121, HW3], f32r)
            s43 = s1s.tile([121, 1], f32, tag="s43")
            nc.scalar.activation(att43[:], l43[:], AF.Exp, accum_out=s43[:])
            rec43 = s1s.tile([121, 1], f32, tag="r43")
            nc.vector.reciprocal(rec43[:], s43[:])
            r4kTs = s1p.tile([121, 256], f32r)
            nc.vector.tensor_scalar_mul(r4kTs[:], r4qkT[:121, 256:512],
                                        rec43[:])

            # s3 = r34 + r3 -> s3pad (f32r), then q3/k3 conv1d(k=3)
            s3pad = s1p.tile([128, 2 * 486], bf16)
            for ct in range(2):
                nc.vector.memset(s3pad[:, ct * 486: ct * 486 + 1], 0.0)
                nc.vector.memset(s3pad[:, ct * 486 + 485: ct * 486 + 486], 0.0)
            for ct in range(2):
                r34 = tmp_ps(128, HW3)
                nc.tensor.matmul(r34[:], r4kTs[:, ct * 128:(ct + 1) * 128],
                                 att43[:], start=True, stop=True)
                nc.vector.tensor_add(
                    s3pad[:, ct * 486 + 1: ct * 486 + 485], r34[:],
                    r3[:, ct * HW3:(ct + 1) * HW3])
            k3 = s1p.tile([128, HW3], f32)
            for qk in range(2):
                pq = tmp_ps(128, HW3)
                for kt in range(2):
                    for tap in range(3):
                        nc.tensor.matmul(
                            pq[:], w13[:, ((kt * 2 + qk) * 3 + tap) * 128:
                                        ((kt * 2 + qk) * 3 + tap + 1) * 128],
                            s3pad[:, kt * 486 + tap: kt * 486 + tap + HW3],
                            start=(kt == 0 and tap == 0),
                            stop=(kt == 1 and tap == 2))
                if qk == 0:
                    nc.scalar.activation(q3[:], pq[:], AF.Identity,
                                         bias=vecs[:, 3:4])
                else:
                    nc.scalar.activation(k3[:], pq[:], AF.Identity,
                                         bias=vecs[:, 4:5])
            for mt in range(4):
                cw = 128 if mt < 3 else 100
                trp = tmp_ps(cw, 128)
                nc.tensor.transpose(trp[:], k3[:, mt * 128: mt * 128 + cw],
                                    ident[:])
                nc.vector.tensor_copy(k3T[:cw, mt * 128:(mt + 1) * 128], trp[:])

        if d.get("DEBUG"):
            nc.sync.dma_start(d["dbg_q3"][:], q3[:].bitcast(f32))
        # ============ PHASE B: att32 -> r23 -> s2 ============
        r23 = ps_big.tile([128, 2048], f32, tag="big")
        MT3 = [(0, 128), (128, 128), (256, 128), (384, 100)]
        prev = None
        for mi in range(len(MT3) + 1):
            if mi < len(MT3):
                q0, mp = MT3[mi]
                att = attp.tile([128, HW2], f32r, tag="att")
                ssum = scr.tile([128, 2], f32, tag="ssum")
                for half in range(2):
                    lg = tmp_ps(128, 1024)
                    for nb in range(2):
                        col = half * 968 + nb * 484
                        nc.tensor.matmul(lg[:mp, nb * 512: nb * 512 + 484],
                                         q3[:, q0: q0 + mp], r2[:, col: col + 484],
                                         start=True, stop=True)
                    lgv = lg[:].rearrange("p (b c) -> p b c", c=512)[:mp, :, :484]
                    nc.scalar.activation(att[:mp, half * 968:(half + 1) * 968],
                                         lgv, AF.Exp,
                                         accum_out=ssum[:mp, half: half + 1])
                s32 = scr.tile([128, 1], f32, tag="s32")
                nc.vector.tensor_add(s32[:mp], ssum[:mp, 0:1], ssum[:mp, 1:2])
                rec = scr.tile([128, 1], f32, tag="rec32")
                nc.vector.reciprocal(rec[:mp], s32[:mp])
                kTs = scr.tile([128, 128], f32r, tag="k3Ts")
                nc.vector.tensor_scalar_mul(kTs[:mp], k3T[:mp, q0: q0 + 128],
                                            rec[:mp])
            if prev is not None:
                pk, pa, pmp, pmi = prev
                for nb in range(4):
                    nc.tensor.matmul(r23[:, nb * 512: nb * 512 + 484], pk[:pmp],
                                     pa[:pmp, nb * 484:(nb + 1) * 484],
                                     start=(pmi == 0), stop=(pmi == len(MT3) - 1))
            if mi < len(MT3):
                prev = (kTs, att, mp, mi)
        for b4 in range(4):
            nc.vector.tensor_add(
                s2pad[:, 1 + b4 * 484: 1 + (b4 + 1) * 484],
                r23[:, b4 * 512: b4 * 512 + 484],
                r2[:, b4 * 484:(b4 + 1) * 484])

        if d.get("DEBUG"):
            nc.sync.dma_start(d["dbg_s2"][:], s2pad[:].bitcast(f32))
        # ===== PHASE C+D interleaved: q2/k2 chunks feed att22 blocks =====
        # (att22 exp work starts as soon as the first q2/k2T block exists,
        # keeping ACT busy under the conv/transpose PE work)
        CH1 = [(0, 484), (484, 484), (968, 484), (1452, 482)]
        r2f = ps_big.tile([128, 2048], f32, tag="big")
        state = {"nT": 0, "nD": 0, "prev": None}

        def emit_transpose(mt):
            cw = 128 if mt < 15 else 14
            trp = tmp_ps(cw, 128)
            nc.tensor.transpose(trp[:], k2[:, mt * 128: mt * 128 + cw], ident[:])
            nc.vector.tensor_copy(k2T[:cw, mt * 128:(mt + 1) * 128], trp[:])

        def emit_r2f_prev():
            pk, pa, pmp, pmt = state["prev"]
            for nb in range(4):
                nc.tensor.matmul(r2f[:, nb * 512: nb * 512 + 484], pk[:pmp],
                                 pa[:pmp, nb * 484:(nb + 1) * 484],
                                 start=(pmt == 0), stop=(pmt == 15))

        def emit_d_iter(mt):
            mp = 128 if mt < 15 else 14
            q0 = mt * 128
            att = attp.tile([128, HW2], f32r, tag="att")
            ssum = scr.tile([128, 2], f32, tag="ssum")
            for half in range(2):
                lg = tmp_ps(128, 1024)
                for nb in range(2):
                    col = half * 968 + nb * 484
                    nc.tensor.matmul(lg[:mp, nb * 512: nb * 512 + 484],
                                     q2[:, q0: q0 + mp], r2[:, col: col + 484],
                                     start=True, stop=True)
                lgv = lg[:].rearrange("p (b c) -> p b c", c=512)[:mp, :, :484]
                nc.scalar.activation(att[:mp, half * 968:(half + 1) * 968],
                                     lgv, AF.Exp,
                                     accum_out=ssum[:mp, half: half + 1])
            s22 = scr.tile([128, 1], f32, tag="s32")
            nc.vector.tensor_add(s22[:mp], ssum[:mp, 0:1], ssum[:mp, 1:2])
            rec = scr.tile([128, 1], f32, tag="rec32")
            nc.vector.reciprocal(rec[:mp], s22[:mp])
            kTs = scr.tile([128, 128], f32r, tag="k3Ts")
            nc.vector.tensor_scalar_mul(kTs[:mp], k2T[:mp, q0: q0 + 128],
                                        rec[:mp])
            if state["prev"] is not None:
                emit_r2f_prev()
            state["prev"] = (kTs, att, mp, mt)

        for sc in range(4):
            st, sz = CH1[sc]
            for qk in range(2):
                pq = tmp_ps(128, sz)
                for tap in range(5):
                    nc.tensor.matmul(
                        pq[:], w12[:, (qk * 5 + tap) * 128:
                                    (qk * 5 + tap + 1) * 128],
                        s2pad[:, st + tap: st + tap + sz],
                        start=(tap == 0), stop=(tap == 4))
                if qk == 0:
                    nc.vector.tensor_scalar_add(q2[:, st: st + sz], pq[:],
                                                vecs[:, 5:6])
                else:
                    nc.vector.tensor_scalar_add(k2[:, st: st + sz], pq[:],
                                                vecs[:, 6:7])
            limit = st + sz
            while state["nT"] < 16:
                mt = state["nT"]
                if mt * 128 + (128 if mt < 15 else 14) > limit:
                    break
                emit_transpose(mt)
                state["nT"] += 1
            while state["nD"] < state["nT"]:
                mt = state["nD"]
                if mt * 128 + (128 if mt < 15 else 14) > limit:
                    break
                emit_d_iter(mt)
                state["nD"] += 1
        while state["nD"] < 16:
            emit_d_iter(state["nD"])
            state["nD"] += 1
        emit_r2f_prev()
        state["prev"] = None
        # r2f -> padded grid interior (4 bank-strided pieces of 11 rows)
        for b4 in range(4):
            nc.vector.tensor_copy(
                _view2d(r2fpad[:, :PG2], 46)[:, 1 + 11 * b4: 12 + 11 * b4, 1:45],
                _view2d(r2f[:, b4 * 512: b4 * 512 + 484], 44))

        # ============ PHASE E: final conv + BN2 + residual ============
        ysums = scr.tile([128, 4], f32, tag="ysums")
        ysqs = scr.tile([128, 4], f32, tag="ysqs")
        CH2 = [(0, 506), (506, 506), (1012, 506), (1518, 506)]
        for ci, (st, sz) in enumerate(CH2):
            pc = tmp_ps(128, sz)
            for tap in range(9):
                dy, dx = tap // 3, tap % 3
                nc.tensor.matmul(
                    pc[:], w2r[:, tap * 128:(tap + 1) * 128],
                    r2fpad[:, st + dy * 46 + dx: st + dy * 46 + dx + sz],
                    start=(tap == 0), stop=(tap == 8))
            r0 = st // 46
            src = _view2d(pc[:, :11 * 46], 46)[:, :, :44]
            nc.scalar.activation(
                y_sb[:, r0 * 44: r0 * 44 + 11 * 44], src,
                AF.Identity, accum_out=ysums[:, ci: ci + 1])
            ysq_s = scr.tile([128, 506], f32, tag="ysq_s")
            nc.scalar.activation(ysq_s[:, :11 * 44], src, AF.Square,
                                 accum_out=ysqs[:, ci: ci + 1])
        if d.get("DEBUG"):
            nc.sync.dma_start(d["dbg_y"][:], y_sb[:])
        stats2 = acts.tile([128, 2], f32)
        nc.vector.reduce_sum(stats2[:, 0:1], ysums[:], axis=AX.X)
        nc.vector.reduce_sum(stats2[:, 1:2], ysqs[:], axis=AX.X)
        nc.sync.dma_start(d["cc2_in"][:], stats2[:].bitcast(f32r))
        nc.gpsimd.collective_compute(
            "AllReduce", ALU.add, replica_groups=d["RG"],
            ins=[d["cc2_in"][:].opt()], outs=[d["cc2_out"][:].opt()])
        gs2 = acts.tile([128, 2], f32)
        nc.sync.dma_start(gs2[:], d["cc2_out"][:].bitcast(f32))
        if d.get("DEBUG"):
            nc.sync.dma_start(d["dbg_gs2"][:], gs2[:])
        c2c = 1.0 / (B * HW2)
        mean2 = acts.tile([128, 1], f32)
        nc.vector.tensor_scalar_mul(mean2[:], gs2[:, 0:1], c2c)
        ex22 = acts.tile([128, 1], f32)
        nc.vector.tensor_scalar_mul(ex22[:], gs2[:, 1:2], c2c)
        var2 = acts.tile([128, 1], f32)
        nc.vector.tensor_mul(var2[:], mean2[:], mean2[:])
        nc.vector.tensor_sub(var2[:], ex22[:], var2[:])
        eps2 = acts.tile([128, 1], f32)
        nc.vector.memset(eps2[:], EPS)
        lnv2 = acts.tile([128, 1], f32)
        nc.scalar.activation(lnv2[:], var2[:], AF.Ln, bias=eps2[:])
        rstd2 = acts.tile([128, 1], f32)
        nc.scalar.activation(rstd2[:], lnv2[:], AF.Exp, scale=-0.5)
        A2 = acts.tile([128, 1], f32)
        nc.vector.tensor_mul(A2[:], vecs[:, 7:8], rstd2[:])
        mA2 = acts.tile([128, 1], f32)
        nc.vector.tensor_mul(mA2[:], mean2[:], A2[:])
        B2 = acts.tile([128, 1], f32)
        nc.vector.tensor_sub(B2[:], vecs[:, 8:9], mA2[:])
        out_sb = attp.tile([128, HW2], f32r, tag="att", name="out_sb")
        for hh in range(4):
            cols = slice(hh * 484, (hh + 1) * 484)
            nc.scalar.activation(out_sb[:, cols], y_sb[:, cols], AF.Relu,
                                 bias=B2[:], scale=A2[:])
            ov = _view2d(out_sb[:, cols], 44)
            nc.vector.tensor_add(
                ov, ov,
                _view2d(xp2[:, :PG2], 46)[:, 1 + 11 * hh: 12 + 11 * hh, 1:45])
            nc.sync.dma_start(d["d_out"][:, cols], out_sb[:, cols].bitcast(f32))


# ---------------- host-side input prep ----------------

def _prep_shared(inputs):
    import ml_dtypes
    g = lambda k: np.ascontiguousarray(np.asarray(inputs[k], dtype=np.float32))
    w4qk = np.empty((4, 128, 9, 512), np.float32)
    wq, wk = g("w_r4q"), g("w_r4k")       # [256, 512, 3, 3]
    for t in range(4):
        ci = slice(t * 128, (t + 1) * 128)
        # [128ci, 3,3, 256co] per source
        w4qk[t, :, :, :256] = wq[:, ci].transpose(1, 2, 3, 0).reshape(128, 9, 256)
        w4qk[t, :, :, 256:] = wk[:, ci].transpose(1, 2, 3, 0).reshape(128, 9, 256)
    w4qk = w4qk.reshape(4, 128, 9 * 512).astype(ml_dtypes.bfloat16)

    wn3s = g("w_n3")                      # [256, 256, 3, 3]
    wn3 = np.empty((2, 128, 2, 9, 128), np.float32)
    for kt in range(2):
        for ct in range(2):
            blk = wn3s[ct * 128:(ct + 1) * 128, kt * 128:(kt + 1) * 128]
            wn3[kt, :, ct] = blk.transpose(1, 2, 3, 0).reshape(128, 9, 128)
    wn3 = wn3.reshape(2, 128, 2304)
    wn3 = np.concatenate([wn3[0], wn3[1]], axis=1)      # [128, 4608]

    wn2 = g("w_n2").transpose(1, 2, 3, 0).reshape(128, 1152)
    w2r = g("w_2r").transpose(1, 2, 3, 0).reshape(128, 1152)

    w13 = np.empty((2, 128, 2, 3, 128), np.float32)
    for kt in range(2):
        for qk, key in enumerate(("w1_3q", "w1_3k")):
            blk = g(key)[:, kt * 128:(kt + 1) * 128]    # [128co, 128ci, 3]
            w13[kt, :, qk] = blk.transpose(1, 2, 0)
    w13 = np.concatenate([w13[0].reshape(128, 768),
                          w13[1].reshape(128, 768)], axis=1)

    w12 = np.empty((128, 2, 5, 128), np.float32)
    for qk, key in enumerate(("w1_2q", "w1_2k")):
        w12[:, qk] = g(key).transpose(1, 2, 0)
    w12 = w12.reshape(128, 1280)

    bn4 = np.stack([np.concatenate([g("g_r4q"), g("g_r4k")]),
                    np.concatenate([g("be_r4q"), g("be_r4k")])])
    vecs = np.zeros((128, 10), np.float32)
    vecs[:, 0] = g("b_n3")[:128]
    vecs[:, 1] = g("b_n3")[128:]
    vecs[:, 2] = g("b_n2")
    vecs[:, 3] = g("b1_3q")
    vecs[:, 4] = g("b1_3k")
    vecs[:, 5] = g("b1_2q")
    vecs[:, 6] = g("b1_2k")
    vecs[:, 7] = g("g_2r")
    vecs[:, 8] = g("be_2r")
    ones = np.ones((128, 128), np.float32)
    bf = ml_dtypes.bfloat16
    bf = ml_dtypes.bfloat16
    return dict(w4qk=w4qk, wn3=wn3.astype(bf), wn2=wn2.astype(bf),
                w2r=w2r.astype(bf), w13=w13.astype(bf), w12=w12.astype(bf),
                bn4=bn4, vecs=vecs, onesd=ones)


def _prep_sample(inputs, i):
    import ml_dtypes
    x4 = np.asarray(inputs["x4"][i], dtype=np.float32)   # [512, 11, 11]
    x4p = np.zeros((512, 13, 13), np.float32)
    x4p[:, 1:12, 1:12] = x4
    xp4s = np.empty((4, 128, 9, 121), np.float32)
    for t in range(4):
        ci = slice(t * 128, (t + 1) * 128)
        for tap in range(9):
            dy, dx = tap // 3, tap % 3
            xp4s[t, :, tap] = x4p[ci, dy:dy + 11, dx:dx + 11].reshape(128, 121)
    xp4s = xp4s.reshape(4, 128, 9 * 121).astype(ml_dtypes.bfloat16)

    x3 = np.asarray(inputs["x3"][i], dtype=np.float32)   # [256, 22, 22]
    x3p = np.zeros((256, 24, 24), np.float32)
    x3p[:, 1:23, 1:23] = x3
    xp3 = np.zeros((2, 128, XP3_W), np.float32)
    xp3[:, :, :PG3] = x3p.reshape(2, 128, PG3)
    xp3 = np.concatenate([xp3[0], xp3[1]], axis=1).astype(ml_dtypes.bfloat16)

    x2 = np.asarray(inputs["x2"][i], dtype=np.float32)   # [128, 44, 44]
    x2p = np.zeros((128, 46, 46), np.float32)
    x2p[:, 1:45, 1:45] = x2
    xp2 = np.zeros((128, XP2_W), np.float32)
    xp2[:, :PG2] = x2p.reshape(128, PG2)
    xp2 = xp2.astype(ml_dtypes.bfloat16)
    return dict(xp4s=xp4s, xp3=xp3, xp2=xp2)


_NC = None
_NC_KEY = None


def _get_nc(shared, debug=False):
    global _NC, _NC_KEY
    key = sum(int(np.abs(np.asarray(v, dtype=np.float32)).sum() * 997)
              for v in shared.values())
    if _NC is None or _NC_KEY != key:
        _NC = build(shared, DEBUG=debug)
        _NC_KEY = key
    return _NC


def run(inputs, trace=False, debug=False):
    shared = _prep_shared(inputs)
    nc = _get_nc(shared, debug)
    in_maps = [_prep_sample(inputs, i) for i in range(N_CORES)]
    last_err = None
    for attempt in range(3):
        try:
            res = run_bass_kernel_spmd(nc, in_maps, list(range(N_CORES)),
                                       trace=trace)
            break
        except Exception as e:  # transient device errors (NRT_EXEC_UNIT etc.)
            last_err = e
            if attempt == 2:
                raise
    out = np.stack([res.results[i]["out"].reshape(128, 44, 44)
                    for i in range(N_CORES)]).astype(np.float32)
    return out, res


def kernel(**inputs):
    out, _ = run(inputs, trace=False)
    return out
